# revision 21
# baseline (speedup 1.0000x reference)
"""Trainium2 Bass kernel for nn_MultiHeadAttention_60816736911814.

Reference semantics (all derived from `src`; `k`/`v` args ignored):
  x  = channel_shuffle(src)          # [B,S,G,C]->[B,S,C,G] flatten, G=5
  xh = split_heads(x)                # [B,H,S,dk], H=16, dk=80
  q/k/v = per-head Linear(dk,dk)     # weights [H,dk,dk] + bias
  attn  = softmax(q kᵀ / sqrt(dk)) v
  out   = concat(attn) @ Woᵀ + bo    # Wo [D,D], D=1280

Sharding (8 cores, no collectives): core i handles batch b=i//2 and query
rows [512*(i%2), 512*(i%2)+512). Each core gets src[b] ROLLED so its query
rows are rows 0..511 (key order is irrelevant to softmax+sum), letting all
cores run an identical program. Wo is applied per-core on its row slice, so
the full output is a pure concatenation.

All matmuls run in bf16 with fp32 PSUM accumulation. The channel shuffle,
head split and Linear biases are folded into host-side weight layout:
 - device-side xhT rows use d' ordering with d = 5*(d'%16) + d'//16, so the
   channels of head h at row d' are exactly src channel 256*(d'//16)+16h+
   (d'%16) -> a contiguous 16-channel strip per (h, r=d'//16), produced by
   plain 128x128 PE transposes of src + one rectangular SBUF->SBUF DMA.
 - projection weights are permuted with the same d' order and get the bias
   appended as contraction row 80 (paired with a ones row 80 in xhT).
 - softmax denominator Z comes free as row 80 of the attention matmul by
   augmenting V with a ones column.
"""

import numpy as np
import ml_dtypes

B, S, D = 4, 1024, 1280
H, DK, G = 16, 80, 5
N_CORES = 8
SH = S // 2  # 512 query rows per core
SCALE = 1.0 / float(np.sqrt(DK))
NT = S // 128  # 8 s-tiles
NCT = D // 128  # 10 channel tiles

_BUILT = {}


def _legalize_waits(nc, mybir):
    """This walrus build allows 1 sync-wait per instruction (2 on
    EventSemaphore). Tile can emit more; split overflow waits onto
    injected same-engine NoOp carriers placed just before the
    instruction (engines run their stream in order -> AND semantics)."""
    n_fix = 0
    for f in nc.m.functions:
        for blk in f.blocks:
            out = []
            changed = False
            for inst in blk.instructions:
                cap = 2 if type(inst).__name__ == "InstEventSemaphore" else 1
                si = inst.sync_info
                if si is not None and si.on_wait and len(si.on_wait) > cap:
                    waits = list(si.on_wait)
                    for w in waits[:-cap]:
                        nop = mybir.InstNoOp(name=f"I-waitfix-{n_fix}")
                        n_fix += 1
                        nop.engine = inst.engine
                        nop.sync_info = mybir.SyncInfo(on_wait=[w], on_update=[])
                        out.append(nop)
                    inst.sync_info = mybir.SyncInfo(
                        on_wait=waits[-cap:], on_update=list(si.on_update)
                    )
                    changed = True
                out.append(inst)
            if changed:
                try:
                    blk.instructions = out
                except Exception:
                    blk.instructions.clear()
                    blk.instructions.extend(out)
    return n_fix


def _build(legalize=True):
    import concourse.bass as bass
    import concourse.mybir as mybir
    import concourse.tile as tile

    f32 = mybir.dt.float32
    bf16 = mybir.dt.bfloat16

    nc = bass.Bass(trn_type="TRN2", target_bir_lowering=False, debug=False)

    x_d = nc.dram_tensor("x", [S, D], bf16, kind="ExternalInput").ap()
    wq_d = nc.dram_tensor("wq", [DK + 1, H, DK], bf16, kind="ExternalInput").ap()
    wk_d = nc.dram_tensor("wk", [DK + 1, H, DK], bf16, kind="ExternalInput").ap()
    wv_d = nc.dram_tensor("wv", [DK + 1, H, DK], bf16, kind="ExternalInput").ap()
    wo_d = nc.dram_tensor("wo", [128, 11, D], bf16, kind="ExternalInput").ap()
    idf_d = nc.dram_tensor("identf", [128, 128], bf16, kind="ExternalInput").ap()
    on2_d = nc.dram_tensor("ones2d", [128, 128], bf16, kind="ExternalInput").ap()
    onr_d = nc.dram_tensor("onesrow", [1, H * S], bf16, kind="ExternalInput").ap()
    on80_d = nc.dram_tensor("ones80", [1, DK], bf16, kind="ExternalInput").ap()
    out_d = nc.dram_tensor("out", [SH, D], f32, kind="ExternalOutput").ap()

    with tile.TileContext(nc) as tc:
        with (
            tc.tile_pool(name="const", bufs=1) as const,
            tc.tile_pool(name="big", bufs=1) as big,
            tc.tile_pool(name="ld", bufs=3) as ld,
            tc.tile_pool(name="et", bufs=8) as etp,
            tc.tile_pool(name="sm", bufs=3) as sm,
            tc.tile_pool(name="ps", bufs=4, space="PSUM") as ps,
        ):
            identf = const.tile([128, 128], bf16)
            nc.scalar.dma_start(out=identf, in_=idf_d)
            on2_sb = const.tile([128, 128], bf16)
            nc.scalar.dma_start(out=on2_sb, in_=on2_d)

            wq_sb = big.tile([DK + 1, H, DK], bf16)
            wk_sb = big.tile([DK + 1, H, DK], bf16)
            wv_sb = big.tile([DK + 1, H, DK], bf16)
            wo_sb = big.tile([128, 11, D], bf16)
            nc.scalar.dma_start(out=wq_sb, in_=wq_d)
            nc.scalar.dma_start(out=wk_sb, in_=wk_d)
            nc.scalar.dma_start(out=wv_sb, in_=wv_d)
            nc.scalar.dma_start(out=wo_sb, in_=wo_d)

            # XH[d', h, s]: transposed shuffled heads (+ ones row 80)
            xh = big.tile([DK + 1, H, S], bf16)
            nc.scalar.dma_start(out=xh[DK : DK + 1, :, :], in_=onr_d)
            # V_ALL[s_in_tile, t*16+h, e(+pad, ones at 96)]
            VW = 97  # Z lands on PSUM partition 96 (32-aligned for engine reads)
            vall = big.tile([128, NT * H, VW], bf16)
            nc.gpsimd.memset(vall[:, :, DK:VW], 1.0)
            # concatT[e, h, q] and K=128-packed ctp[j%128, j//128, q]
            ct = big.tile([DK + 1, H, SH], bf16)
            ctp = big.tile([128, 11, SH], bf16)
            nc.scalar.dma_start(out=ctp[0:1, 10, :], in_=onr_d[:, 0:SH])

            # ---- Stage 1: load src, cast, transpose (c-outer), repack ----
            xt = big.tile([128, NCT, S], bf16)  # x transposed [c, ct, s]
            sbs = []
            for t in range(NT):
                s_f = ld.tile([128, D], bf16, tag="sf", bufs=NT)
                nc.sync.dma_start(out=s_f, in_=x_d[t * 128 : (t + 1) * 128, :])
                sbs.append(s_f)
            rep = 0
            for c in [0, 2, 4, 6, 8, 1, 3, 5, 7, 9]:
                for t in range(NT):
                    p_ps = ps.tile([128, 128], bf16, tag="rot", bufs=3)
                    nc.tensor.transpose(p_ps, sbs[t][:, c * 128 : (c + 1) * 128], identf)
                    nc.vector.tensor_copy(xt[:, c, t * 128 : (t + 1) * 128], p_ps)
                r = c // 2
                eng = [nc.gpsimd, nc.sync, nc.scalar][rep % 3]
                rep += 1
                for h in range(8 * (c % 2), 8 * (c % 2) + 8):
                    poff = 16 * (h % 8)
                    eng.dma_start(
                        out=xh[16 * r : 16 * r + 16, h, :],
                        in_=xt[poff : poff + 16, c, :],
                    )

            # ---- Stage 3: V projections (t-outer, head-grouped) ----
            groups = [list(range(0, 6)), [6, 7], list(range(8, 14)), [14, 15]]
            for t in range(NT):
                for grp in groups:
                    ng = len(grp)
                    vp = ps.tile([128, 6, DK], f32, tag="rot", bufs=3)
                    for i, h in enumerate(grp):
                        nc.tensor.matmul(
                            vp[:, i, :],
                            xh[:, h, t * 128 : (t + 1) * 128],
                            wv_sb[:, h, :],
                            start=True,
                            stop=True,
                        )
                    nc.vector.tensor_copy(
                        vall[:, t * H + grp[0] : t * H + grp[0] + ng, 0:DK],
                        vp[:, 0:ng, :],
                    )

            # ---- Stage 4: projections + attention per head ----
            grp_state = {"zg": None, "pend": []}

            def _normalize_group(grp_state=grp_state):
                zg = grp_state["zg"]
                HQ = SH // 2
                brs = {}
                for half in range(2):
                    qs = slice(half * HQ, half * HQ + HQ)
                    zr = sm.tile([128, HQ], f32, tag="zr", bufs=2, name="zr")
                    nc.vector.reciprocal(zr, zg[:, qs])
                    zrb = sm.tile([128, HQ], bf16, tag="zrb", bufs=2, name="zrb")
                    nc.vector.tensor_copy(zrb, zr)
                    for k, (hh, hu) in enumerate(grp_state["pend"]):
                        br_ps = brs.get(k)
                        if br_ps is None:
                            br_ps = ps.tile(
                                [DK, SH], f32, tag="br", bufs=1, name="br_ps"
                            )
                            brs[k] = br_ps
                        nc.tensor.matmul(
                            br_ps[:, qs], on2_sb[32 * k : 32 * k + 1, 0:DK],
                            zrb[32 * k : 32 * k + 1, :],
                            start=True, stop=True,
                            tile_position=(32 * k, 0),
                        )
                        nc.vector.tensor_mul(
                            ct[0:DK, hh, qs], hu[:, qs], br_ps[:, qs]
                        )
                for k, (hh, hu) in enumerate(grp_state["pend"]):
                    j0 = DK * hh
                    pl, off = j0 // 128, j0 % 128
                    l1 = min(128 - off, DK)
                    nc.gpsimd.dma_start(
                        out=ctp[off : off + l1, pl, :], in_=ct[0:l1, hh, :]
                    )
                    if l1 < DK:
                        nc.sync.dma_start(
                            out=ctp[0 : DK - l1, pl + 1, :], in_=ct[l1:DK, hh, :]
                        )
                grp_state["zg"] = None
                grp_state["pend"] = []

            for h in range(H):
                qt_ps = ps.tile([DK, SH], f32, tag="qk", bufs=2)
                nc.tensor.matmul(
                    qt_ps, wq_sb[:, h, :], xh[:, h, 0:SH], start=True, stop=True
                )
                qt_sb = sm.tile([DK, SH], bf16, tag="qt", bufs=2)
                nc.vector.tensor_copy(qt_sb, qt_ps)
                kt_sb = sm.tile([DK, S], bf16, tag="kt", bufs=2)
                for j in range(2):
                    kt_ps = ps.tile([DK, SH], f32, tag="qk", bufs=2)
                    nc.tensor.matmul(
                        kt_ps,
                        wk_sb[:, h, :],
                        xh[:, h, j * SH : (j + 1) * SH],
                        start=True,
                        stop=True,
                    )
                    nc.vector.tensor_copy(kt_sb[:, j * SH : (j + 1) * SH], kt_ps)

                hz_ps = ps.tile([VW, SH], f32, tag="hz", bufs=2)
                ets = []
                for t in range(NT):
                    sc_ps = ps.tile([128, SH], f32, tag="rot", bufs=3)
                    nc.tensor.matmul(
                        sc_ps,
                        kt_sb[:, t * 128 : (t + 1) * 128],
                        qt_sb,
                        start=True,
                        stop=True,
                    )
                    et = etp.tile([128, SH], bf16, tag="et")
                    nc.scalar.activation(
                        et, sc_ps, mybir.ActivationFunctionType.Exp, scale=SCALE
                    )
                    ets.append(et)
                for t in range(NT):
                    nc.tensor.matmul(
                        hz_ps,
                        vall[:, t * H + h, :],
                        ets[t],
                        start=(t == 0),
                        stop=(t == NT - 1),
                    )
                if grp_state["zg"] is None:
                    grp_state["zg"] = sm.tile(
                        [128, SH], f32, tag="zg", bufs=1, name="zg"
                    )
                k = len(grp_state["pend"])
                nc.scalar.copy(
                    grp_state["zg"][32 * k : 32 * k + 1, :], hz_ps[VW - 1 : VW, :]
                )
                hu = sm.tile([DK, SH], bf16, tag="hu", bufs=4, name="hu")
                nc.vector.tensor_copy(hu, hz_ps[0:DK, :])
                grp_state["pend"].append((h, hu))
                if len(grp_state["pend"]) == 4:
                    _normalize_group()

            # ---- Stage 5: output projection ----
            ocuts = [(0, 512), (512, 1024), (1024, 1280)]
            for qt in range(SH // 128):
                for o0, o1 in ocuts:
                    op = ps.tile([128, 512], f32, tag="rot", bufs=3)
                    for jt in range(11):
                        kh = 1 if jt == 10 else 128
                        nc.tensor.matmul(
                            op[:, 0 : o1 - o0],
                            ctp[0:kh, jt, qt * 128 : (qt + 1) * 128],
                            wo_sb[0:kh, jt, o0:o1],
                            start=(jt == 0),
                            stop=(jt == 10),
                        )
                    o_sb = sm.tile([128, 512], f32, tag="osb", bufs=2)
                    nc.vector.tensor_copy(o_sb[:, 0 : o1 - o0], op[:, 0 : o1 - o0])
                    nc.gpsimd.dma_start(
                        out=out_d[qt * 128 : (qt + 1) * 128, o0:o1],
                        in_=o_sb[:, 0 : o1 - o0],
                    )

    if legalize:
        _legalize_waits(nc, mybir)
    return nc


def _host_prep(Wq, bq, Wk, bk, Wv, bv, Wo, bo):
    bf = ml_dtypes.bfloat16
    dprime = np.arange(DK)
    perm = 5 * (dprime % 16) + dprime // 16  # d' -> d

    def aug(Wx, bx):
        # [H, e, d] -> [H, d', e] permuted, + bias row -> [dk+1, H, dk]
        wt = Wx.transpose(0, 2, 1)[:, perm, :]  # [H, d', e]
        a = np.concatenate([wt, bx[:, None, :]], axis=1)  # [H, dk+1, dk]
        return np.ascontiguousarray(a.transpose(1, 0, 2)).astype(bf)

    wq = aug(Wq, bq)
    wk = aug(Wk, bk)
    wv = aug(Wv, bv)

    wo_t = np.concatenate([Wo.T, np.zeros((128 * 11 - D, D), np.float32)])
    wo_t[D] = bo  # row 0 of plane 10, paired with the ones row in ctp
    wo = np.ascontiguousarray(
        wo_t.reshape(11, 128, D).transpose(1, 0, 2)
    ).astype(bf)

    consts = {
        "identf": np.eye(128, dtype=bf),
        "ones2d": np.ones((128, 128), bf),
        "onesrow": np.ones((1, H * S), bf),
        "ones80": np.ones((1, DK), bf),
    }
    return wq, wk, wv, wo, consts


def kernel(**inputs):
    from concourse.bass_utils import run_bass_kernel_spmd

    src = np.asarray(inputs["src"], np.float32)
    wq, wk, wv, wo, consts = _host_prep(
        np.asarray(inputs["Wq"], np.float32),
        np.asarray(inputs["bq"], np.float32),
        np.asarray(inputs["Wk"], np.float32),
        np.asarray(inputs["bk"], np.float32),
        np.asarray(inputs["Wv"], np.float32),
        np.asarray(inputs["bv"], np.float32),
        np.asarray(inputs["Wo"], np.float32),
        np.asarray(inputs["bo"], np.float32),
    )

    if "nc" not in _BUILT:
        _BUILT["nc"] = _build()
    nc = _BUILT["nc"]

    in_maps = []
    for i in range(N_CORES):
        b, qlo = i // 2, (i % 2) * SH
        x = np.roll(src[b], -qlo, axis=0)
        in_maps.append(
            {
                "x": np.ascontiguousarray(x).astype(ml_dtypes.bfloat16),
                "wq": wq,
                "wk": wk,
                "wv": wv,
                "wo": wo,
                **consts,
            }
        )

    res = run_bass_kernel_spmd(nc, in_maps, core_ids=list(range(N_CORES)))

    out = np.empty((B, S, D), np.float32)
    for i in range(N_CORES):
        b, qlo = i // 2, (i % 2) * SH
        out[b, qlo : qlo + SH] = res.results[i]["out"]
    return out


# revision 25
# speedup vs baseline: 1.0523x; 1.0523x over previous
"""Trainium2 Bass kernel for nn_MultiHeadAttention_60816736911814.

Reference semantics (all derived from `src`; `k`/`v` args ignored):
  x  = channel_shuffle(src)          # [B,S,G,C]->[B,S,C,G] flatten, G=5
  xh = split_heads(x)                # [B,H,S,dk], H=16, dk=80
  q/k/v = per-head Linear(dk,dk)     # weights [H,dk,dk] + bias
  attn  = softmax(q kᵀ / sqrt(dk)) v
  out   = concat(attn) @ Woᵀ + bo    # Wo [D,D], D=1280

Sharding (8 cores, no collectives): core i handles batch b=i//2 and query
rows [512*(i%2), 512*(i%2)+512). Each core gets src[b] ROLLED so its query
rows are rows 0..511 (key order is irrelevant to softmax+sum), letting all
cores run an identical program. Wo is applied per-core on its row slice, so
the full output is a pure concatenation.

All matmuls run in bf16 with fp32 PSUM accumulation. The channel shuffle,
head split and Linear biases are folded into host-side weight layout:
 - device-side xhT rows use d' ordering with d = 5*(d'%16) + d'//16, so the
   channels of head h at row d' are exactly src channel 256*(d'//16)+16h+
   (d'%16) -> a contiguous 16-channel strip per (h, r=d'//16), produced by
   plain 128x128 PE transposes of src + one rectangular SBUF->SBUF DMA.
 - projection weights are permuted with the same d' order and get the bias
   appended as contraction row 80 (paired with a ones row 80 in xhT).
 - softmax denominator Z comes free as row 80 of the attention matmul by
   augmenting V with a ones column.
"""

import numpy as np
import ml_dtypes

B, S, D = 4, 1024, 1280
H, DK, G = 16, 80, 5
N_CORES = 8
SH = S // 2  # 512 query rows per core
SCALE = 1.0 / float(np.sqrt(DK))
NT = S // 128  # 8 s-tiles
NCT = D // 128  # 10 channel tiles

_BUILT = {}


def _legalize_waits(nc, mybir):
    """This walrus build allows 1 sync-wait per instruction (2 on
    EventSemaphore). Tile can emit more; split overflow waits onto
    injected same-engine NoOp carriers placed just before the
    instruction (engines run their stream in order -> AND semantics)."""
    n_fix = 0
    for f in nc.m.functions:
        for blk in f.blocks:
            out = []
            changed = False
            for inst in blk.instructions:
                cap = 2 if type(inst).__name__ == "InstEventSemaphore" else 1
                si = inst.sync_info
                if si is not None and si.on_wait and len(si.on_wait) > cap:
                    waits = list(si.on_wait)
                    for w in waits[:-cap]:
                        nop = mybir.InstNoOp(name=f"I-waitfix-{n_fix}")
                        n_fix += 1
                        nop.engine = inst.engine
                        nop.sync_info = mybir.SyncInfo(on_wait=[w], on_update=[])
                        out.append(nop)
                    inst.sync_info = mybir.SyncInfo(
                        on_wait=waits[-cap:], on_update=list(si.on_update)
                    )
                    changed = True
                out.append(inst)
            if changed:
                try:
                    blk.instructions = out
                except Exception:
                    blk.instructions.clear()
                    blk.instructions.extend(out)
    return n_fix


def _build(legalize=True):
    import concourse.bass as bass
    import concourse.mybir as mybir
    import concourse.tile as tile

    f32 = mybir.dt.float32
    bf16 = mybir.dt.bfloat16

    nc = bass.Bass(trn_type="TRN2", target_bir_lowering=False, debug=False)

    x_d = nc.dram_tensor("x", [S, D], bf16, kind="ExternalInput").ap()
    wq_d = nc.dram_tensor("wq", [DK + 1, H, DK], bf16, kind="ExternalInput").ap()
    wk_d = nc.dram_tensor("wk", [DK + 1, H, DK], bf16, kind="ExternalInput").ap()
    wv_d = nc.dram_tensor("wv", [DK + 1, H, DK], bf16, kind="ExternalInput").ap()
    wo_d = nc.dram_tensor("wo", [128, 11, D], bf16, kind="ExternalInput").ap()
    idf_d = nc.dram_tensor("identf", [128, 128], bf16, kind="ExternalInput").ap()
    on2_d = nc.dram_tensor("ones2d", [128, 128], bf16, kind="ExternalInput").ap()
    onr_d = nc.dram_tensor("onesrow", [1, H * S], bf16, kind="ExternalInput").ap()
    on80_d = nc.dram_tensor("ones80", [1, DK], bf16, kind="ExternalInput").ap()
    out_d = nc.dram_tensor("out", [SH, D], f32, kind="ExternalOutput").ap()

    with tile.TileContext(nc) as tc:
        with (
            tc.tile_pool(name="const", bufs=1) as const,
            tc.tile_pool(name="big", bufs=1) as big,
            tc.tile_pool(name="ld", bufs=3) as ld,
            tc.tile_pool(name="et", bufs=8) as etp,
            tc.tile_pool(name="sm", bufs=3) as sm,
            tc.tile_pool(name="ps", bufs=4, space="PSUM") as ps,
        ):
            identf = const.tile([128, 128], bf16)
            nc.scalar.dma_start(out=identf, in_=idf_d)
            on2_sb = const.tile([128, 128], bf16)
            nc.scalar.dma_start(out=on2_sb, in_=on2_d)

            wq_sb = big.tile([DK + 1, H, DK], bf16)
            wk_sb = big.tile([DK + 1, H, DK], bf16)
            wv_sb = big.tile([DK + 1, H, DK], bf16)
            wo_sb = big.tile([128, 11, D], bf16)
            nc.scalar.dma_start(out=wq_sb, in_=wq_d)
            nc.scalar.dma_start(out=wk_sb, in_=wk_d)
            nc.scalar.dma_start(out=wv_sb, in_=wv_d)
            nc.scalar.dma_start(out=wo_sb, in_=wo_d)

            # XH[d', h, s]: transposed shuffled heads (+ ones row 80)
            xh = big.tile([DK + 1, H, S], bf16)
            nc.scalar.dma_start(out=xh[DK : DK + 1, :, :], in_=onr_d)
            VW = 97  # Z lands on PSUM partition 96 (32-aligned for engine reads)
            # concatT[e, h, q] and K=128-packed ctp[j%128, j//128, q]
            ct = big.tile([DK + 1, H, SH], bf16)
            ctp = big.tile([128, 11, SH], bf16)
            nc.scalar.dma_start(out=ctp[0:1, 10, :], in_=onr_d[:, 0:SH])

            # ---- Stage 1: load src, cast, transpose (c-outer), repack ----
            xt = big.tile([128, NCT, S], bf16)  # x transposed [c, ct, s]
            sbs = []
            for t in range(NT):
                s_f = ld.tile([128, D], bf16, tag="sf", bufs=NT)
                nc.sync.dma_start(out=s_f, in_=x_d[t * 128 : (t + 1) * 128, :])
                sbs.append(s_f)
            rep = 0
            for c in [0, 2, 4, 6, 8, 1, 3, 5, 7, 9]:
                for t in range(NT):
                    p_ps = ps.tile([128, 128], bf16, tag="rot", bufs=2)
                    nc.tensor.transpose(p_ps, sbs[t][:, c * 128 : (c + 1) * 128], identf)
                    nc.vector.tensor_copy(xt[:, c, t * 128 : (t + 1) * 128], p_ps)
                r = c // 2
                eng = [nc.gpsimd, nc.sync, nc.scalar][rep % 3]
                rep += 1
                for h in range(8 * (c % 2), 8 * (c % 2) + 8):
                    poff = 16 * (h % 8)
                    eng.dma_start(
                        out=xh[16 * r : 16 * r + 16, h, :],
                        in_=xt[poff : poff + 16, c, :],
                    )

            # ---- Stage 4: projections + attention per head ----
            grp_state = {"zg": None, "pend": []}

            def _normalize_group(grp_state=grp_state):
                zg = grp_state["zg"]
                ng = len(grp_state["pend"])
                zr = sm.tile([128, SH], f32, tag="zr", bufs=1, name="zr")
                nc.vector.reciprocal(zr, zg)
                zrb = sm.tile([128, SH], bf16, tag="zrb", bufs=2, name="zrb")
                nc.vector.tensor_copy(zrb, zr)
                for k, (hh, hu) in enumerate(grp_state["pend"]):
                    br_ps = ps.tile(
                        [DK, SH], f32, tag="br", bufs=1, name="br_ps"
                    )
                    nc.tensor.matmul(
                        br_ps, on2_sb[32 * k : 32 * k + 1, 0:DK],
                        zrb[32 * k : 32 * k + 1, :],
                        start=True, stop=True,
                        tile_position=(32 * k, 0),
                    )
                    nc.vector.tensor_mul(ct[0:DK, hh, :], hu, br_ps)
                    j0 = DK * hh
                    pl, off = j0 // 128, j0 % 128
                    l1 = min(128 - off, DK)
                    nc.gpsimd.dma_start(
                        out=ctp[off : off + l1, pl, :], in_=ct[0:l1, hh, :]
                    )
                    if l1 < DK:
                        nc.sync.dma_start(
                            out=ctp[0 : DK - l1, pl + 1, :], in_=ct[l1:DK, hh, :]
                        )
                grp_state["zg"] = None
                grp_state["pend"] = []

            for h in range(H):
                vh = sm.tile([128, NT, VW], bf16, tag="vh", bufs=3, name="vh")
                nc.gpsimd.memset(vh[:, :, DK:VW], 1.0)
                for half in range(2):
                    vp = ps.tile(
                        [128, NT // 2, DK], f32, tag="vp", bufs=2, name="vp"
                    )
                    for u in range(NT // 2):
                        t = half * (NT // 2) + u
                        nc.tensor.matmul(
                            vp[:, u, :],
                            xh[:, h, t * 128 : (t + 1) * 128],
                            wv_sb[:, h, :],
                            start=True,
                            stop=True,
                        )
                    nc.vector.tensor_copy(
                        vh[:, half * (NT // 2) : (half + 1) * (NT // 2), 0:DK], vp
                    )

                qt_ps = ps.tile([DK, SH], f32, tag="qk", bufs=1)
                nc.tensor.matmul(
                    qt_ps, wq_sb[:, h, :], xh[:, h, 0:SH], start=True, stop=True
                )
                qt_sb = sm.tile([DK, SH], bf16, tag="qt", bufs=2)
                nc.vector.tensor_copy(qt_sb, qt_ps)
                kt_sb = sm.tile([DK, S], bf16, tag="kt", bufs=2)
                for j in range(2):
                    kt_ps = ps.tile([DK, SH], f32, tag="qk", bufs=1)
                    nc.tensor.matmul(
                        kt_ps,
                        wk_sb[:, h, :],
                        xh[:, h, j * SH : (j + 1) * SH],
                        start=True,
                        stop=True,
                    )
                    nc.vector.tensor_copy(kt_sb[:, j * SH : (j + 1) * SH], kt_ps)

                hz_ps = ps.tile([VW, SH], f32, tag="hz", bufs=2)
                ets = []
                for t in range(NT):
                    sc_ps = ps.tile([128, SH], f32, tag="rot", bufs=2)
                    nc.tensor.matmul(
                        sc_ps,
                        kt_sb[:, t * 128 : (t + 1) * 128],
                        qt_sb,
                        start=True,
                        stop=True,
                    )
                    et = etp.tile([128, SH], bf16, tag="et")
                    nc.scalar.activation(
                        et, sc_ps, mybir.ActivationFunctionType.Exp, scale=SCALE
                    )
                    ets.append(et)
                for t in range(NT):
                    nc.tensor.matmul(
                        hz_ps,
                        vh[:, t, :],
                        ets[t],
                        start=(t == 0),
                        stop=(t == NT - 1),
                    )
                if grp_state["zg"] is None:
                    grp_state["zg"] = sm.tile(
                        [128, SH], f32, tag="zg", bufs=1, name="zg"
                    )
                k = len(grp_state["pend"])
                nc.scalar.copy(
                    grp_state["zg"][32 * k : 32 * k + 1, :], hz_ps[VW - 1 : VW, :]
                )
                hu = sm.tile([DK, SH], bf16, tag="hu", bufs=4, name="hu")
                nc.vector.tensor_copy(hu, hz_ps[0:DK, :])
                grp_state["pend"].append((h, hu))
                if len(grp_state["pend"]) == 4:
                    _normalize_group()

            # ---- Stage 5: output projection ----
            ocuts = [(0, 512), (512, 1024), (1024, 1280)]
            for qt in range(SH // 128):
                for o0, o1 in ocuts:
                    op = ps.tile([128, 512], f32, tag="rot", bufs=2)
                    for jt in range(11):
                        kh = 1 if jt == 10 else 128
                        nc.tensor.matmul(
                            op[:, 0 : o1 - o0],
                            ctp[0:kh, jt, qt * 128 : (qt + 1) * 128],
                            wo_sb[0:kh, jt, o0:o1],
                            start=(jt == 0),
                            stop=(jt == 10),
                        )
                    o_sb = sm.tile([128, 512], f32, tag="osb", bufs=2)
                    nc.vector.tensor_copy(o_sb[:, 0 : o1 - o0], op[:, 0 : o1 - o0])
                    nc.gpsimd.dma_start(
                        out=out_d[qt * 128 : (qt + 1) * 128, o0:o1],
                        in_=o_sb[:, 0 : o1 - o0],
                    )

    if legalize:
        _legalize_waits(nc, mybir)
    return nc


def _host_prep(Wq, bq, Wk, bk, Wv, bv, Wo, bo):
    bf = ml_dtypes.bfloat16
    dprime = np.arange(DK)
    perm = 5 * (dprime % 16) + dprime // 16  # d' -> d

    def aug(Wx, bx):
        # [H, e, d] -> [H, d', e] permuted, + bias row -> [dk+1, H, dk]
        wt = Wx.transpose(0, 2, 1)[:, perm, :]  # [H, d', e]
        a = np.concatenate([wt, bx[:, None, :]], axis=1)  # [H, dk+1, dk]
        return np.ascontiguousarray(a.transpose(1, 0, 2)).astype(bf)

    wq = aug(Wq, bq)
    wk = aug(Wk, bk)
    wv = aug(Wv, bv)

    wo_t = np.concatenate([Wo.T, np.zeros((128 * 11 - D, D), np.float32)])
    wo_t[D] = bo  # row 0 of plane 10, paired with the ones row in ctp
    wo = np.ascontiguousarray(
        wo_t.reshape(11, 128, D).transpose(1, 0, 2)
    ).astype(bf)

    consts = {
        "identf": np.eye(128, dtype=bf),
        "ones2d": np.ones((128, 128), bf),
        "onesrow": np.ones((1, H * S), bf),
        "ones80": np.ones((1, DK), bf),
    }
    return wq, wk, wv, wo, consts


def kernel(**inputs):
    from concourse.bass_utils import run_bass_kernel_spmd

    src = np.asarray(inputs["src"], np.float32)
    wq, wk, wv, wo, consts = _host_prep(
        np.asarray(inputs["Wq"], np.float32),
        np.asarray(inputs["bq"], np.float32),
        np.asarray(inputs["Wk"], np.float32),
        np.asarray(inputs["bk"], np.float32),
        np.asarray(inputs["Wv"], np.float32),
        np.asarray(inputs["bv"], np.float32),
        np.asarray(inputs["Wo"], np.float32),
        np.asarray(inputs["bo"], np.float32),
    )

    if "nc" not in _BUILT:
        _BUILT["nc"] = _build()
    nc = _BUILT["nc"]

    in_maps = []
    for i in range(N_CORES):
        b, qlo = i // 2, (i % 2) * SH
        x = np.roll(src[b], -qlo, axis=0)
        in_maps.append(
            {
                "x": np.ascontiguousarray(x).astype(ml_dtypes.bfloat16),
                "wq": wq,
                "wk": wk,
                "wv": wv,
                "wo": wo,
                **consts,
            }
        )

    res = run_bass_kernel_spmd(nc, in_maps, core_ids=list(range(N_CORES)))

    out = np.empty((B, S, D), np.float32)
    for i in range(N_CORES):
        b, qlo = i // 2, (i % 2) * SH
        out[b, qlo : qlo + SH] = res.results[i]["out"]
    return out
